# revision 45
# baseline (speedup 1.0000x reference)
"""Cached multi-head attention on 8 TRN2 NeuronCores.

Sharding: core c = 2*b + g handles batch b (of 4) and head-group g (of 2,
8 heads each) -- data parallel on batch x tensor parallel on heads.
Column-parallel Wq/Wk/Wv, row-parallel Wo; the Wo all-reduce (sum of the
two head-group partials per batch) is done on host during the unshard,
along with the bo bias add.

Device schedule (per core): the exp on the Scalar engine (~1.1us per
128x1024 score block) and the PE (~218us of matmul streaming) are kept
continuously busy by emitting attention blocks as a flat stream (ST_k
issued, PV_{k-LAG} trailing) with projection / out-projection matmuls
queued as small fill closures consumed one per block slot.  All weights
and activations are pre-swizzled on the host into SBUF layout so every
DMA is a contiguous 128-descriptor transfer, and startup DMAs are
ordered most-critical-first across the three DGE queues.

Causal masks get a fast path: blocks above the diagonal are skipped,
diagonal blocks use shortened matmuls + gpsimd affine_select zeroing.
Arbitrary masks fall back to per-block skip/plain/mixed classification
with host-shipped multiplicative mask tiles.
"""

import bisect
import math

import ml_dtypes
import numpy as np

import concourse.bass as bass
import concourse.mybir as mybir
import concourse.tile as tile
from concourse import bacc
from concourse.bass_utils import run_bass_kernel_spmd

F32 = mybir.dt.float32
BF16 = mybir.dt.bfloat16
AF = mybir.ActivationFunctionType
ts = bass.ts

B, T, D, H = 4, 2048, 1024, 16
HD = D // H          # 64
NCORE = 8
DG = D // 2          # 512 dims per core (8 heads)
NPAIR = 4            # head pairs per core
SB = 128             # s-block size
TC = 512             # attention t-chunk
NTC = T // TC        # 4
NSB = T // SB        # 16
PC = 512             # projection t-chunk (x streaming granularity)
CCH = D // 128       # 8 contraction chunks

LAG = 4              # PV trails ST by this many blocks

_cache = {}
last_result = {}


def _classify_blocks(mask):
    """Per (s_blk, t_chunk) classification, unioned across batches (SPMD)."""
    causal = np.triu(np.ones((T, T), dtype=bool), k=1)
    if all(np.array_equal(mask[b], causal) for b in range(B)):
        return "causal", None, None
    cls = np.zeros((NSB, NTC), dtype=np.int64)
    for s in range(NSB):
        for i in range(NTC):
            per_b_all = [mask[b, i * TC:(i + 1) * TC, s * SB:(s + 1) * SB].all()
                         for b in range(B)]
            per_b_any = [mask[b, i * TC:(i + 1) * TC, s * SB:(s + 1) * SB].any()
                         for b in range(B)]
            if all(per_b_all):
                cls[s, i] = 0
            elif not any(per_b_any):
                cls[s, i] = 1
            else:
                cls[s, i] = 2
    mixed = [(s, i) for s in range(NSB) for i in range(NTC) if cls[s, i] == 2]
    return "general", cls, mixed


def _build(mode, cls, n_mixed):
    nc = bacc.Bacc("TRN2", target_bir_lowering=False, debug=False,
                   num_devices=NCORE)
    d = {}
    # host pre-swizzled layouts: every DMA is contiguous per partition
    for nm in ("xq", "xk", "xv"):
        d[nm] = nc.dram_tensor(nm, [128, NTC, CCH, PC], BF16,
                               kind="ExternalInput").ap()
    for nm in ("wq", "wk"):
        d[nm] = nc.dram_tensor(nm, [128, NPAIR, CCH * 128], BF16,
                               kind="ExternalInput").ap()
    d["wv"] = nc.dram_tensor("wv", [128, CCH * DG], BF16, kind="ExternalInput").ap()
    d["wo"] = nc.dram_tensor("wo", [128, NPAIR * D], BF16, kind="ExternalInput").ap()
    d["bq"] = nc.dram_tensor("bq", [128, NPAIR], F32, kind="ExternalInput").ap()
    d["bk"] = nc.dram_tensor("bk", [128, NPAIR], F32, kind="ExternalInput").ap()
    d["bv"] = nc.dram_tensor("bv", [1, DG], BF16, kind="ExternalInput").ap()
    d["ones1"] = nc.dram_tensor("ones1", [1, 128], BF16, kind="ExternalInput").ap()
    if n_mixed:
        d["mmask"] = nc.dram_tensor("mmask", [n_mixed, SB, TC], BF16,
                                    kind="ExternalInput").ap()
    out_d = nc.dram_tensor("out", [T, D], F32, kind="ExternalOutput").ap()

    with tile.TileContext(nc) as tc:
        with (
            tc.tile_pool(name="persist", bufs=1) as pp,
            tc.tile_pool(name="stream", bufs=2) as sp,
            tc.tile_pool(name="small", bufs=2) as mp,
            tc.tile_pool(name="psum", bufs=2, space="PSUM") as psp,
        ):
            HV = HD + 1  # 65: V columns + ones column per head

            # ---- persistent tiles --------------------------------------
            wv_sb = pp.tile([128, CCH * DG], BF16, tag="wv")
            wq_sb = pp.tile([128, NPAIR * CCH * 128], BF16, tag="wq")
            wk_sb = pp.tile([128, NPAIR * CCH * 128], BF16, tag="wk")
            wo_sb = pp.tile([128, NPAIR * D], BF16, tag="wo")
            bq_sb = pp.tile([128, NPAIR], F32, tag="bq")
            bk_sb = pp.tile([128, NPAIR], F32, tag="bk")
            bv_sb = pp.tile([1, DG], BF16, tag="bv")
            ones1_sb = pp.tile([1, 128], BF16, tag="ones1")
            v_sb = [pp.tile([128, 8 * HV], BF16, tag=f"v{s}", name=f"v{s}")
                    for s in range(NSB)]
            w_sb = {"wq": wq_sb, "wk": wk_sb}

            # ---- startup DMAs: balance the three ~100GB/s DGE queues ---
            # scalar (Act) HWDGE queue: pair-0 weights + xk0 + smalls
            PW = CCH * 128  # columns per pair in wq/wk sbuf layout
            nc.scalar.dma_start(out=wq_sb[:, 0:PW], in_=d["wq"][:, 0])
            nc.scalar.dma_start(out=wk_sb[:, 0:PW], in_=d["wk"][:, 0])
            # gpsimd: warmup tile memset first (gates PE warmup)
            warm_sb = pp.tile([128, 512], BF16, tag="warm")
            nc.gpsimd.memset(warm_sb[:], 1.0)

            # x chunk streaming
            x_tiles = {}
            _xq_rr = [0]

            def load_x(kind, tau, eng=None):
                if (kind, tau) in x_tiles:
                    return
                if eng is None:
                    eng = (nc.sync, nc.gpsimd)[_xq_rr[0] % 2]
                    _xq_rr[0] += 1
                xx = sp.tile([128, CCH * PC], BF16, tag=kind, bufs=2,
                             name=f"{kind}{tau}")
                eng.dma_start(
                    out=xx[:].rearrange("p (c t) -> p c t", t=PC),
                    in_=d[kind][:, tau])
                x_tiles[(kind, tau)] = xx

            nc.scalar.dma_start(out=bv_sb[:], in_=d["bv"][:])
            nc.scalar.dma_start(out=ones1_sb[:], in_=d["ones1"][:])
            nc.scalar.dma_start(out=bq_sb[:], in_=d["bq"][:])
            nc.scalar.dma_start(out=bk_sb[:], in_=d["bk"][:])
            # sync: xq0 then xv0; gpsimd: xk0 then wv
            load_x("xq", 0, nc.sync)
            load_x("xk", 0, nc.gpsimd)
            load_x("xv", 0, nc.sync)
            nc.gpsimd.dma_start(out=wv_sb[:], in_=d["wv"][:])
            for s_ in range(NSB):
                ones_cols = v_sb[s_][:].rearrange("p (h c) -> p h c", c=HV)[:, :, HD:HV]
                nc.gpsimd.memset(ones_cols, 1.0)
            # remaining weight pairs on scalar (smalls already queued)
            for p_ in (1, 2, 3):
                nc.scalar.dma_start(out=wq_sb[:, p_ * PW:(p_ + 1) * PW],
                                    in_=d["wq"][:, p_])
                nc.scalar.dma_start(out=wk_sb[:, p_ * PW:(p_ + 1) * PW],
                                    in_=d["wk"][:, p_])

            # PE warmup: ramp the p-state while startup DMAs land; results
            # are discarded
            for _ in range(20):
                wps = psp.tile([128, TC], F32, tag="b512", bufs=2, name="wps")
                nc.tensor.matmul(wps[:], warm_sb[:, 0:128], warm_sb[:],
                                 start=True, stop=True)

            qT = [[pp.tile([128, TC], BF16, tag=f"qT{p}_{i}", name=f"qT{p}_{i}")
                   for i in range(NTC)] for p in range(NPAIR)]
            kT = [[pp.tile([128, TC], BF16, tag=f"kT{p}_{i}", name=f"kT{p}_{i}")
                   for i in range(NTC)] for p in range(NPAIR)]
            oT = [[pp.tile([128, TC], BF16, tag=f"oT{p}_{i}", name=f"oT{p}_{i}")
                   for i in range(NTC)] for p in range(NPAIR)]

            # ---- fill closures -----------------------------------------
            def v_fills(sigma):
                tau, u = sigma // 4, sigma % 4
                st_ = {}

                def a():
                    load_x("xv", tau)
                    x = x_tiles[("xv", tau)]
                    ps = psp.tile([128, TC], F32, tag="b512", bufs=2)
                    for c in range(4):
                        nc.tensor.matmul(
                            ps[:], x[:, c * PC + u * SB:c * PC + (u + 1) * SB],
                            wv_sb[:, ts(c, DG)], start=(c == 0), stop=False)
                    st_["ps"] = ps

                def b():
                    x = x_tiles[("xv", tau)]
                    ps = st_.pop("ps")
                    for c in range(4, CCH):
                        nc.tensor.matmul(
                            ps[:], x[:, c * PC + u * SB:c * PC + (u + 1) * SB],
                            wv_sb[:, ts(c, DG)], start=False, stop=False)
                    nc.tensor.matmul(ps[:], ones1_sb[:], bv_sb[:],
                                     start=False, stop=True)
                    vdst = v_sb[sigma][:].rearrange("p (h c) -> p h c", c=HV)[:, :, 0:HD]
                    nc.vector.tensor_copy(vdst, ps[:].rearrange("p (h c) -> p h c", c=HD))

                return [a, b]

            def qk_fills(nm, p, i):
                st_ = {}
                dst = qT if nm == "q" else kT
                bias = bq_sb if nm == "q" else bk_sb
                xkind = "xq" if nm == "q" else "xk"
                w = w_sb["w" + nm]

                def a():
                    load_x(xkind, i)
                    xx = x_tiles[(xkind, i)]
                    ps = psp.tile([128, TC], F32, tag="b512", bufs=2)
                    for c in range(4):
                        nc.tensor.matmul(
                            ps[:], w[:, (p * CCH + c) * 128:(p * CCH + c + 1) * 128],
                            xx[:, ts(c, PC)], start=(c == 0), stop=False)
                    st_["ps"] = ps

                def b():
                    xx = x_tiles[(xkind, i)]
                    ps = st_.pop("ps")
                    for c in range(4, CCH):
                        nc.tensor.matmul(
                            ps[:], w[:, (p * CCH + c) * 128:(p * CCH + c + 1) * 128],
                            xx[:, ts(c, PC)], start=False, stop=(c == CCH - 1))
                    nc.vector.tensor_scalar(
                        out=dst[p][i][:], in0=ps[:],
                        scalar1=bias[:, p:p + 1], scalar2=None,
                        op0=mybir.AluOpType.add)

                return [a, b]

            def outproj_fills(i, tt):
                # chunks 0..2: full 4-pair accumulation per (tt, e)
                st_ = {}

                def a():
                    ps = psp.tile([128, TC], F32, tag="b512", bufs=2, name="ops")
                    for p in range(NPAIR):
                        nc.tensor.matmul(
                            ps[:], oT[p][i][:, ts(tt - 4 * i, 128)],
                            wo_sb[:, p * D + 0 * TC:p * D + 1 * TC],
                            start=(p == 0), stop=(p == NPAIR - 1))
                    st_["ps0"] = ps

                def b():
                    ob = mp.tile([128, D], F32, tag="ob", bufs=4, name="ob")
                    st_["ob"] = ob
                    nc.vector.tensor_copy(ob[:, ts(0, TC)], st_.pop("ps0")[:])
                    ps = psp.tile([128, TC], F32, tag="b512", bufs=2, name="ops")
                    for p in range(NPAIR):
                        nc.tensor.matmul(
                            ps[:], oT[p][i][:, ts(tt - 4 * i, 128)],
                            wo_sb[:, p * D + 1 * TC:p * D + 2 * TC],
                            start=(p == 0), stop=(p == NPAIR - 1))
                    st_["ps1"] = ps

                def c():
                    ob = st_.pop("ob")
                    nc.vector.tensor_copy(ob[:, ts(1, TC)], st_.pop("ps1")[:])
                    (nc.sync, nc.gpsimd)[tt % 2].dma_start(
                        out=out_d[ts(tt, 128), :], in_=ob[:])

                return [a, b, c]

            # last chunk: pairs 0-2 accumulated during the chunk, pair 3 +
            # store in the tail (keeps the tail to 8 matmuls + adds)
            last_ob = {}

            def outproj_partial_fills(i, tt):
                def mk(e):
                    def fn():
                        ps = psp.tile([128, TC], F32, tag="b512", bufs=2, name="opp")
                        for p in range(3):
                            nc.tensor.matmul(
                                ps[:], oT[p][i][:, ts(tt - 4 * i, 128)],
                                wo_sb[:, p * D + e * TC:p * D + (e + 1) * TC],
                                start=(p == 0), stop=(p == 2))
                        if tt not in last_ob:
                            last_ob[tt] = mp.tile([128, D], F32, tag="ob",
                                                  bufs=4, name="obL")
                        nc.vector.tensor_copy(last_ob[tt][:, ts(e, TC)], ps[:])
                    return fn
                return [mk(0), mk(1)]

            def outproj_tail(i, tt):
                ob = last_ob[tt]
                ps = psp.tile([128, 2 * TC], F32, tag="stAB", bufs=2, name="opt")
                for e in range(2):
                    nc.tensor.matmul(
                        ps[:, ts(e, TC)], oT[3][i][:, ts(tt - 4 * i, 128)],
                        wo_sb[:, 3 * D + e * TC:3 * D + (e + 1) * TC],
                        start=True, stop=True)
                nc.vector.tensor_add(ob[:], ps[:], ob[:])
                (nc.sync, nc.gpsimd, nc.scalar)[tt % 3].dma_start(
                    out=out_d[ts(tt, 128), :], in_=ob[:])

            # fill queue: (marker, seq, fn) kept sorted by marker; fills with
            # marker <= u are forced before unit u's first ST.  99 =
            # pump-only (tail-drained).
            fills = []
            _fseq = [0]

            def fpush(marker, fn):
                bisect.insort(fills, (marker, _fseq[0], fn))
                _fseq[0] += 1

            def drain_until(u):
                while fills and fills[0][0] <= u:
                    fills.pop(0)[2]()

            def pump_slot(u):
                # always emit due fills; spend pump-only fills (markers>=90)
                # only once the last chunk is near, where the PE runs dry
                if fills and (fills[0][0] <= u + 1 or u >= 11):
                    fills.pop(0)[2]()

            for um in range(1, NTC * NPAIR):
                i, p = um // 4, um % 4
                if p == 0 and i > 0:
                    for sg in range(4 * i, 4 * i + 4):
                        for fn in v_fills(sg):
                            fpush(um, fn)
                for nm in ("q", "k"):
                    for fn in qk_fills(nm, p, i):
                        fpush(um, fn)

            # ---- attention unit construction ---------------------------
            scale = 1.0 / math.sqrt(HD)

            def build_unit(i, p):
                if mode == "causal":
                    blocks = []
                    for s_blk in range(4 * i + 4):
                        j = s_blk - 4 * i
                        if j < 0:
                            blocks.append((s_blk, i * TC, TC, False))
                        else:
                            s0 = SB * s_blk
                            toff = s0 if j < 3 else s0 - SB
                            blocks.append((s_blk, toff, TC * (i + 1) - toff, True))
                else:
                    blocks = [(s_blk, i * TC, TC, False)
                              for s_blk in range(NSB) if cls[s_blk, i] != 0]
                state = {"p2": {}, "ot": None}

                def make_st(bi):
                    s_blk, toff, n, diag = blocks[bi]

                    def fn():
                        s0 = SB * s_blk
                        sc, lo = s_blk // 4, SB * (s_blk % 4)
                        tl = toff - i * TC
                        st2 = psp.tile([128, 2 * TC], F32, tag="stAB", bufs=2,
                                       name="st2")
                        nc.tensor.matmul(
                            st2[:, 0:n], kT[p][sc][0:HD, lo:lo + SB],
                            qT[p][i][0:HD, tl:tl + n],
                            start=True, stop=True, tile_position=(0, 0))
                        nc.tensor.matmul(
                            st2[:, TC:TC + n], kT[p][sc][HD:128, lo:lo + SB],
                            qT[p][i][HD:128, tl:tl + n],
                            start=True, stop=True, tile_position=(64, 0))
                        p2 = sp.tile([128, 2 * TC], BF16, tag="pAB", bufs=8,
                                     name="p2")
                        if n == TC:
                            nc.scalar.activation(p2[:], st2[:], AF.Exp, scale=scale)
                        else:
                            st3 = st2[:].rearrange("p (b c) -> p b c", b=2)[:, :, 0:n]
                            p3 = p2[:].rearrange("p (b c) -> p b c", b=2)[:, :, 0:n]
                            nc.scalar.activation(p3, st3, AF.Exp, scale=scale)
                        if mode == "causal" and diag:
                            w_ = s0 + SB - toff
                            for off in (0, TC):
                                nc.gpsimd.affine_select(
                                    out=p2[:, off:off + w_], in_=p2[:, off:off + w_],
                                    compare_op=mybir.AluOpType.is_ge,
                                    fill=0.0, base=toff - s0,
                                    pattern=[[1, w_]], channel_multiplier=-1)
                        elif mode == "general" and cls[s_blk, i] == 2:
                            mmt = sp.tile([SB, TC], BF16, tag="mmask", name="mmt")
                            nc.sync.dma_start(out=mmt[:],
                                              in_=d["mmask"][mixed_idx[(s_blk, i)]])
                            for off in (0, TC):
                                nc.vector.tensor_mul(p2[:, off:off + n],
                                                     p2[:, off:off + n], mmt[:, 0:n])
                        state["p2"][bi] = p2
                    return fn

                def make_pv(bi):
                    s_blk, toff, n, diag = blocks[bi]

                    def fn():
                        if state["ot"] is None:
                            state["ot"] = (
                                psp.tile([HV, TC], F32, tag="ot", bufs=2, name="otA"),
                                psp.tile([HV, TC], F32, tag="ot", bufs=2, name="otB"))
                        otA, otB = state["ot"]
                        p2 = state["p2"].pop(bi)
                        tl = toff - i * TC
                        vv = v_sb[s_blk][:].rearrange("p (h c) -> p h c", c=HV)
                        first, last = bi == 0, bi == len(blocks) - 1
                        nc.tensor.matmul(otA[:, tl:tl + n], vv[:, 2 * p, :],
                                         p2[:, 0:n], start=first, stop=last)
                        nc.tensor.matmul(otB[:, tl:tl + n], vv[:, 2 * p + 1, :],
                                         p2[:, TC:TC + n], start=first, stop=last)
                    return fn

                def epi_a():
                    otA, otB = state["ot"]
                    lastu = (i == NTC - 1 and p == NPAIR - 1)
                    dq = nc.scalar if lastu else (nc.sync, nc.gpsimd)[p % 2]
                    # copy PSUM out up-front so the banks free quickly; use
                    # the (idle) Act engine except in the Act-bound last chunk
                    cpA = mp.tile([HV, TC], F32, tag="ocp", bufs=4, name="cpA")
                    cpB = mp.tile([HV, TC], F32, tag="ocp", bufs=4, name="cpB")
                    if i < NTC - 1:
                        nc.scalar.activation(cpA[:], otA[:], AF.Copy)
                        nc.scalar.activation(cpB[:], otB[:], AF.Copy)
                    else:
                        nc.vector.tensor_copy(cpA[:], otA[:])
                        nc.vector.tensor_copy(cpB[:], otB[:])
                    # denominators (psum row 64) -> partition 0 -> reciprocal
                    dden = mp.tile([1, 2 * TC], F32, tag="dden", bufs=2,
                                   name="dden")
                    dq.dma_start(out=dden[:, 0:TC], in_=cpA[HD:HV, :])
                    dq.dma_start(out=dden[:, TC:], in_=cpB[HD:HV, :])
                    nc.vector.reciprocal_approx_fast(out=dden[:], in_=dden[:])
                    state["epi"] = (cpA, cpB, dden)

                def epi_b():
                    # runs a few slots later so nothing downstream stalls;
                    # broadcast on gpsimd keeps the PE out of the chain
                    cpA, cpB, dden = state.pop("epi")
                    lastu = (i == NTC - 1 and p == NPAIR - 1)
                    dq = nc.scalar if lastu else (nc.sync, nc.gpsimd)[p % 2]
                    rbd = mp.tile([HD, 2 * TC], F32, tag="rbd", bufs=1,
                                  name="rbd")
                    nc.gpsimd.partition_broadcast(rbd[:], dden[:], channels=HD)
                    nc.vector.tensor_mul(oT[p][i][0:HD, :], cpA[0:HD, :],
                                         rbd[:, 0:TC])
                    stg = mp.tile([HD, TC], BF16, tag="stg", bufs=4,
                                  name="stg")
                    nc.vector.tensor_mul(stg[:], cpB[0:HD, :], rbd[:, TC:])
                    dq.dma_start(out=oT[p][i][HD:128, :], in_=stg[:])

                n = len(blocks)
                return ([make_st(b) for b in range(n)],
                        [make_pv(b) for b in range(n)], epi_a, epi_b)

            # ---- prologue: pair-0 q/k only (v(0..3) woven into slots) --
            for nm in ("q", "k"):
                for fn in qk_fills(nm, 0, 0):
                    fn()
            if mode != "causal":
                for sg in range(NSB):
                    for fn in v_fills(sg):
                        fn()

            # ---- flat block stream with LAG ----------------------------
            units = [(i, p) for i in range(NTC) for p in range(NPAIR)]
            stream = []
            epis = {}
            epis_b = {}
            for u, (i, p) in enumerate(units):
                st_fns, pv_fns, epi_a, epi_b = build_unit(i, p)
                epis[u] = epi_a
                epis_b[u] = epi_b
                nb = len(st_fns)
                for b in range(nb):
                    stream.append((st_fns[b], pv_fns[b], u, b == nb - 1))

            def after_chunk_loads(i):
                if i + 2 < NTC:
                    for kind in ("xv", "xq", "xk"):
                        load_x(kind, i + 2)

            def stage_outproj(i):
                # defer out-projection into the Act-bound last chunk, where
                # the PE otherwise runs dry (markers 90+: pump-only)
                for tt in range(4 * i, 4 * i + 4):
                    for fn in outproj_fills(i, tt):
                        fpush(90 + i, fn)

            def stage_partials():
                for tt in range(4 * (NTC - 1), 4 * NTC):
                    for fn in outproj_partial_fills(NTC - 1, tt):
                        fpush(99, fn)

            nblocks = len(stream)
            wo_issued = [False]
            pend_b = []
            for k in range(nblocks + LAG):
                while pend_b and pend_b[0][0] <= k:
                    pend_b.pop(0)[1]()
                if k < nblocks:
                    stf, _, u, _ = stream[k]
                    drain_until(u)
                    stf()
                if k == 2 and not wo_issued[0]:
                    wo_issued[0] = True
                    nc.scalar.dma_start(out=wo_sb[:], in_=d["wo"][:])
                    for kind in ("xv", "xq", "xk"):
                        load_x(kind, 1)
                pump_slot(stream[min(k, nblocks - 1)][2])
                if mode == "causal" and 4 <= k < 8:
                    for fn in v_fills(k - 4):
                        fn()
                j = k - LAG
                if j >= 0:
                    _, pvf, u, last = stream[j]
                    pvf()
                    if last:
                        epis[u]()
                        pend_b.append((k + 4, epis_b[u]))
                        i, p = units[u]
                        if p == NPAIR - 1 and i < NTC - 1:
                            after_chunk_loads(i)
                            pend_b.append((k + 5, lambda i=i: stage_outproj(i)))
                        if u == NTC * NPAIR - 2:
                            pend_b.append((k + 5, stage_partials))
            for _, fn in pend_b:
                fn()
            while fills:
                fills.pop(0)[2]()
            for tt in range(4 * (NTC - 1), 4 * NTC):
                outproj_tail(NTC - 1, tt)

    nc.compile()
    return nc


def kernel(**inputs):
    query = np.asarray(inputs["query"], np.float32)
    key = np.asarray(inputs["key"], np.float32)
    value = np.asarray(inputs["value"], np.float32)
    mask = np.asarray(inputs["mask"], bool)
    Wq, bq = np.asarray(inputs["Wq"], np.float32), np.asarray(inputs["bq"], np.float32)
    Wk, bk = np.asarray(inputs["Wk"], np.float32), np.asarray(inputs["bk"], np.float32)
    Wv, bv = np.asarray(inputs["Wv"], np.float32), np.asarray(inputs["bv"], np.float32)
    Wo, bo = np.asarray(inputs["Wo"], np.float32), np.asarray(inputs["bo"], np.float32)

    mode, cls, mixed = _classify_blocks(mask)
    global mixed_idx
    if mode == "general":
        mixed_idx = {blk: n for n, blk in enumerate(mixed)}
        n_mixed = len(mixed)
    else:
        mixed_idx, n_mixed = {}, 0

    key_sig = (mode, tuple(cls.ravel()) if cls is not None else None)
    if key_sig not in _cache:
        _cache[key_sig] = _build(mode, cls, n_mixed)
    nc = _cache[key_sig]

    def xswz(x):
        # [T, D] activation -> [128, NTC, CCH, PC] (chunk-contig per partition)
        xT = np.ascontiguousarray(x.T).astype(ml_dtypes.bfloat16)
        return np.ascontiguousarray(
            xT.reshape(CCH, 128, NTC, PC).transpose(1, 2, 0, 3))

    def wswz_qk(W, sl):
        # [DG, D] shard -> transpose -> [128, NPAIR, CCH*128] pair-contig
        WT = np.ascontiguousarray(W[sl, :].T).astype(ml_dtypes.bfloat16)
        return np.ascontiguousarray(
            WT.reshape(CCH, 128, NPAIR, 128).transpose(1, 2, 0, 3).reshape(
                128, NPAIR, CCH * 128))

    in_maps = []
    xs = {}
    for b in range(B):
        xs[("xq", b)] = xswz(query[b])
        xs[("xk", b)] = xswz(key[b])
        xs[("xv", b)] = xswz(value[b])
    for core in range(NCORE):
        b, g = core // 2, core % 2
        sl = slice(g * DG, (g + 1) * DG)
        WvT = np.ascontiguousarray(Wv[sl, :].T).astype(ml_dtypes.bfloat16)
        WoT = np.ascontiguousarray(Wo[:, sl].T).astype(ml_dtypes.bfloat16)
        im = {
            "xq": xs[("xq", b)], "xk": xs[("xk", b)], "xv": xs[("xv", b)],
            "wq": wswz_qk(Wq, sl),
            "wk": wswz_qk(Wk, sl),
            "wv": np.ascontiguousarray(
                WvT.reshape(CCH, 128, DG).transpose(1, 0, 2).reshape(128, CCH * DG)),
            "wo": np.ascontiguousarray(
                WoT.reshape(NPAIR, 128, D).transpose(1, 0, 2).reshape(128, NPAIR * D)),
            "bq": np.ascontiguousarray(bq[sl].reshape(NPAIR, 128).T),
            "bk": np.ascontiguousarray(bk[sl].reshape(NPAIR, 128).T),
            "bv": np.ascontiguousarray(bv[sl])[None, :].astype(ml_dtypes.bfloat16),
            "ones1": np.ones((1, 128), ml_dtypes.bfloat16),
        }
        if n_mixed:
            mm = np.empty((n_mixed, SB, TC), ml_dtypes.bfloat16)
            for n, (s_blk, i) in enumerate(mixed):
                blk = mask[b, i * TC:(i + 1) * TC, s_blk * SB:(s_blk + 1) * SB]
                mm[n] = (~blk.T).astype(np.float32)
            im["mmask"] = mm
        in_maps.append(im)

    r = run_bass_kernel_spmd(nc, in_maps, core_ids=list(range(NCORE)))
    last_result["exec_time_ns"] = r.exec_time_ns
    last_result["profile_json"] = getattr(r, "profile_json", None)
    last_result["instructions_and_trace"] = getattr(r, "instructions_and_trace", None)
    out = np.empty((B, T, D), np.float32)
    for b in range(B):
        out[b] = r.results[2 * b]["out"] + r.results[2 * b + 1]["out"]
    out += bo[None, None, :]
    return out
